# revision 3
# baseline (speedup 1.0000x reference)
"""Per-env MLP (EnvironVectorField) Trainium2 kernel.

Reference computation (fp32):
    x = u.reshape(B, E, D)  # B=16384, E=8 envs, D=64
    h = swish(x @ W1[e] + b1[e]); h = swish(h @ W2[e] + b2[e])
    h = swish(h @ W3[e] + b3[e]); out = h @ W4[e] + b4[e]
    return out.reshape(B*E, D)

Sharding: expert-parallel — core e computes env e entirely (u rows e::8).

Per-core layout: activations kept feature-major (features on SBUF
partitions, batch on the free axis) so weights are the matmul stationary
operand and each weight tile is reused across the whole batch. Input /
output tiles are transposed on the tensor engine via identity matmuls.
Matmuls run in float32r (rounded fp32, ~1e-4 rel err) which streams at
1 cycle/row; fp32 would be 4x slower.
"""

import sys

sys.path.insert(0, '/opt/trn_rl_repo')

import numpy as np

import concourse.bacc as bacc
import concourse.bass as bass
import concourse.mybir as mybir
import concourse.tile as tile
from concourse.bass_utils import run_bass_kernel_spmd
from concourse.masks import make_identity

F32 = mybir.dt.float32
F32R = mybir.dt.float32r
SILU = mybir.ActivationFunctionType.Silu
IDENT = mybir.ActivationFunctionType.Identity

N_ENV = 8
D = 64          # state dim
H = 1024        # hidden dim
B = 16384       # rows per env
NB = 1024       # batch-chunk columns processed per chunk
NCH = B // NB   # 16 chunks
NT = NB // 512  # 512-wide matmul n-tiles per chunk
KT = H // 128   # 8 k/m tiles of 128 over the hidden dim


def build_module(iters: int = 1):
    """Build + compile the SPMD single-core Bass module (same on all 8 cores)."""
    nc = bacc.Bacc("TRN2", target_bir_lowering=False, num_devices=N_ENV)

    xin = nc.dram_tensor("x", (B, D), F32, kind="ExternalInput")
    w1 = nc.dram_tensor("w1", (128, H), F32, kind="ExternalInput")       # [Kpad, M]
    w2 = nc.dram_tensor("w2", (128, KT, H), F32, kind="ExternalInput")   # [ki, ko, M]
    w3 = nc.dram_tensor("w3", (128, KT, H), F32, kind="ExternalInput")
    w4 = nc.dram_tensor("w4", (128, KT, D), F32, kind="ExternalInput")
    b1 = nc.dram_tensor("b1", (128, KT), F32, kind="ExternalInput")      # [mi, mo]
    b2 = nc.dram_tensor("b2", (128, KT), F32, kind="ExternalInput")
    b3 = nc.dram_tensor("b3", (128, KT), F32, kind="ExternalInput")
    b4 = nc.dram_tensor("b4", (D, 1), F32, kind="ExternalInput")
    yout = nc.dram_tensor("y", (B, D), F32, kind="ExternalOutput")

    xv = xin.rearrange("(c j p) d -> c p j d", p=128, j=KT)
    yv = yout.rearrange("(c j p) d -> c p j d", p=128, j=KT)

    from contextlib import ExitStack
    with tile.TileContext(nc) as tc, ExitStack() as ctx:
        wpool = ctx.enter_context(tc.tile_pool(name="wpool", bufs=1))
        wstage = ctx.enter_context(tc.tile_pool(name="wstage", bufs=1))
        stage = ctx.enter_context(tc.tile_pool(name="stage", bufs=3))
        mps = ctx.enter_context(tc.tile_pool(name="mps", bufs=2, space="PSUM"))
        lps = ctx.enter_context(tc.tile_pool(name="lps", bufs=1, space="PSUM"))

        ident = wpool.tile([128, 128], F32)
        make_identity(nc, ident)

        # --- load weights, cast to f32r once ---
        def load_wr(dram, shape, nm):
            tmp = wstage.tile(list(shape), F32, tag="wtmp", name=f"wtmp_{nm}")
            nc.sync.dma_start(tmp[:], dram[:])
            wr = wpool.tile(list(shape), F32R, name=f"{nm}_r")
            nc.vector.tensor_copy(wr[:], tmp[:])
            return wr

        w1r = load_wr(w1, (128, H), "w1")
        w2r = load_wr(w2, (128, KT, H), "w2")
        w3r = load_wr(w3, (128, KT, H), "w3")
        w4r = load_wr(w4, (128, KT, D), "w4")

        b1s = wpool.tile([128, KT], F32)
        b2s = wpool.tile([128, KT], F32)
        b3s = wpool.tile([128, KT], F32)
        b4s = wpool.tile([D, 1], F32)
        nc.sync.dma_start(b1s[:], b1[:])
        nc.sync.dma_start(b2s[:], b2[:])
        nc.sync.dma_start(b3s[:], b3[:])
        nc.sync.dma_start(b4s[:], b4[:])

        # --- persistent activation buffers ---
        xT0 = wpool.tile([128, NB], F32R)
        xT1 = wpool.tile([128, NB], F32R)
        zsrc = wstage.tile([128, NB], F32, tag="wtmp", name="zsrc")
        nc.any.memzero(zsrc[:])
        nc.vector.tensor_copy(xT0[:], zsrc[:])
        nc.vector.tensor_copy(xT1[:], zsrc[:])
        hA = wpool.tile([128, KT, NB], F32R)
        hB = wpool.tile([128, KT, NB], F32R)
        oT0 = wpool.tile([D, NB], F32)
        oT1 = wpool.tile([D, NB], F32)

        def chunk_body(c, it=0):
            u = f"{it}_{c}"
            xT = (xT0, xT1)[c % 2]
            oT = (oT0, oT1)[c % 2]

            xb = stage.tile([128, KT, D], F32, tag="xb", name=f"xb_{u}")
            nc.sync.dma_start(xb[:], xv[c])
            for j in range(KT):
                tp = mps.tile([D, 128], F32, tag="tp", name=f"tp_{u}_{j}")
                nc.tensor.transpose(tp[:], xb[:, j, :], ident[:])
                nc.vector.tensor_copy(xT[:D, j * 128:(j + 1) * 128], tp[:])

            # L1: xT -> hA   (K=128 zero-padded, lhsT = w1r m-tile)
            for m in range(KT):
                for n in range(NT):
                    pm = mps.tile([128, 512], F32, tag=f"mm{n}", name=f"p1_{u}_{m}_{n}")
                    nc.tensor.matmul(pm[:], w1r[:, m * 128:(m + 1) * 128],
                                     xT[:, n * 512:(n + 1) * 512],
                                     start=True, stop=True)
                    nc.scalar.activation(hA[:, m, n * 512:(n + 1) * 512], pm[:],
                                         SILU, bias=b1s[:, m:m + 1])

            # L2: hA -> hB, L3: hB -> hA
            for li, (wr, bs, hs, hd) in enumerate(
                    ((w2r, b2s, hA, hB), (w3r, b3s, hB, hA))):
                for m in range(KT):
                    pms = [mps.tile([128, 512], F32, tag=f"mm{n}",
                                    name=f"p{li + 2}_{u}_{m}_{n}") for n in range(NT)]
                    for k in range(KT):
                        for n in range(NT):
                            nc.tensor.matmul(pms[n][:], wr[:, k, m * 128:(m + 1) * 128],
                                             hs[:, k, n * 512:(n + 1) * 512],
                                             start=(k == 0), stop=(k == KT - 1))
                    for n in range(NT):
                        nc.scalar.activation(hd[:, m, n * 512:(n + 1) * 512], pms[n][:],
                                             SILU, bias=bs[:, m:m + 1])

            # L4: hA -> oT  (M=64)
            for n in range(NT):
                p4 = lps.tile([D, 512], F32, tag=f"l4{n}", name=f"p4_{u}_{n}")
                for k in range(KT):
                    nc.tensor.matmul(p4[:], w4r[:, k, :],
                                     hA[:, k, n * 512:(n + 1) * 512],
                                     start=(k == 0), stop=(k == KT - 1))
                nc.scalar.activation(oT[:, n * 512:(n + 1) * 512], p4[:],
                                     IDENT, bias=b4s[:, 0:1])

            # transpose back to batch-major and store
            ob = stage.tile([128, KT, D], F32, tag="ob", name=f"ob_{u}")
            for j in range(KT):
                tq = mps.tile([128, D], F32, tag="tp", name=f"tq_{u}_{j}")
                nc.tensor.transpose(tq[:], oT[:, j * 128:(j + 1) * 128], ident[:D, :D])
                nc.vector.tensor_copy(ob[:, j, :], tq[:])
            nc.sync.dma_start(yv[c], ob[:])

        if iters == 1:
            for c in range(NCH):
                chunk_body(c)
        else:
            with tc.For_i(0, iters, 1):
                for c in range(NCH):
                    chunk_body(c)

    nc.compile()
    return nc


def _prep_in_maps(t, u, W1, b1, W2, b2, W3, b3, W4, b4):
    in_maps = []
    for e in range(N_ENV):
        w1p = np.zeros((128, H), np.float32)
        w1p[:D] = W1[e]
        in_maps.append({
            "x": np.ascontiguousarray(u[e::N_ENV]),
            "w1": w1p,
            "w2": np.ascontiguousarray(W2[e].reshape(KT, 128, H).transpose(1, 0, 2)),
            "w3": np.ascontiguousarray(W3[e].reshape(KT, 128, H).transpose(1, 0, 2)),
            "w4": np.ascontiguousarray(W4[e].reshape(KT, 128, D).transpose(1, 0, 2)),
            "b1": np.ascontiguousarray(b1[e].reshape(KT, 128).T),
            "b2": np.ascontiguousarray(b2[e].reshape(KT, 128).T),
            "b3": np.ascontiguousarray(b3[e].reshape(KT, 128).T),
            "b4": np.ascontiguousarray(b4[e].reshape(D, 1)),
        })
    return in_maps


_CACHED_NC = None


def kernel(t, u, W1, b1, W2, b2, W3, b3, W4, b4):
    global _CACHED_NC
    u = np.asarray(u, np.float32)
    args = [np.asarray(a, np.float32) for a in (W1, b1, W2, b2, W3, b3, W4, b4)]
    if _CACHED_NC is None:
        _CACHED_NC = build_module()
    in_maps = _prep_in_maps(None, u, *args)
    res = run_bass_kernel_spmd(_CACHED_NC, in_maps, core_ids=list(range(N_ENV)))
    out = np.empty((B * N_ENV, D), np.float32)
    for e in range(N_ENV):
        out[e::N_ENV] = res.results[e]["y"]
    return out


# revision 14
# speedup vs baseline: 5.8097x; 5.8097x over previous
"""Per-env MLP (EnvironVectorField) Trainium2 kernel.

Reference computation (fp32):
    x = u.reshape(B, E, D)  # B=16384, E=8 envs, D=64
    h = swish(x @ W1[e] + b1[e]); h = swish(h @ W2[e] + b2[e])
    h = swish(h @ W3[e] + b3[e]); out = h @ W4[e] + b4[e]
    return out.reshape(B*E, D)

Sharding: expert-parallel — core e computes env e entirely (u rows e::8).

Per-core design: activations are kept feature-major (features on SBUF
partitions, batch on the free axis) so weights are the stationary matmul
operand and every weight tile is reused across the whole batch. Input and
output tiles are transposed on the tensor engine via identity matmuls.
Matmuls run in float32r (rounded fp32, ~1e-4 rel err), which streams at
1 cycle/row; full fp32 would be 4x slower.

The batch is processed in chunks of NB columns. Layer 1 of chunk c+1 is
interleaved into layer 3 of chunk c (three rotating h buffers) because
layer 1 has only 2 matmuls per PSUM evacuation and would otherwise stall
the tensor engine behind the scalar engine's Silu evacuations.
"""

import sys

sys.path.insert(0, '/opt/trn_rl_repo')

from contextlib import ExitStack

import numpy as np

import concourse.bacc as bacc
import concourse.bass as bass
import concourse.mybir as mybir
import concourse.tile as tile
from concourse.bass_utils import run_bass_kernel_spmd
from concourse.masks import make_identity

F32 = mybir.dt.float32
F32R = mybir.dt.float32r
SILU = mybir.ActivationFunctionType.Silu
IDENT = mybir.ActivationFunctionType.Identity

N_ENV = 8
D = 64          # state dim
H = 1024        # hidden dim
B = 16384       # rows per env
NB = 1024       # batch-chunk columns per chunk
NCH = B // NB   # 16 chunks
NT = NB // 512  # 512-wide matmul n-tiles per chunk
KT = H // 128   # 8 k/m tiles of 128 over the hidden dim


def build_module(iters: int = 1):
    nc = bacc.Bacc("TRN2", target_bir_lowering=False, num_devices=N_ENV)

    xin = nc.dram_tensor("x", (B, D), F32, kind="ExternalInput")
    w1 = nc.dram_tensor("w1", (128, H), F32, kind="ExternalInput")       # [Kpad, M]
    w2 = nc.dram_tensor("w2", (128, KT, H), F32, kind="ExternalInput")   # [ki, ko, M]
    w3 = nc.dram_tensor("w3", (128, KT, H), F32, kind="ExternalInput")
    w4 = nc.dram_tensor("w4", (128, KT, D), F32, kind="ExternalInput")
    b1 = nc.dram_tensor("b1", (128, KT), F32, kind="ExternalInput")      # [mi, mo]
    b2 = nc.dram_tensor("b2", (128, KT), F32, kind="ExternalInput")
    b3 = nc.dram_tensor("b3", (128, KT), F32, kind="ExternalInput")
    b4 = nc.dram_tensor("b4", (D, 1), F32, kind="ExternalInput")
    yout = nc.dram_tensor("y", (B, D), F32, kind="ExternalOutput")

    xv = xin.rearrange("(c j p) d -> c p j d", p=128, j=KT)
    yv = yout.rearrange("(c j p) d -> c p j d", p=128, j=KT)

    with tile.TileContext(nc) as tc, ExitStack() as ctx:
        wpool = ctx.enter_context(tc.tile_pool(name="wpool", bufs=1))
        wstage = ctx.enter_context(tc.tile_pool(name="wstage", bufs=2))
        mps = ctx.enter_context(tc.tile_pool(name="mps", bufs=3, space="PSUM"))
        tpp = ctx.enter_context(tc.tile_pool(name="tpp", bufs=2, space="PSUM"))

        ident = wpool.tile([128, 128], F32)
        make_identity(nc, ident)

        # xT zero-fill first: rows D..127 stay zero (K=64 padded to 128)
        xT0 = wpool.tile([128, NB], F32R)
        xT1 = wpool.tile([128, NB], F32R)
        zsrc = wstage.tile([128, 2, H], F32, tag="wtmp", name="zsrc")
        nc.any.memzero(zsrc[:])
        nc.vector.tensor_copy(xT0[:], zsrc[:, 0, :])
        nc.vector.tensor_copy(xT1[:], zsrc[:, 0, :])

        # biases in one padded tile
        ball = wpool.tile([128, 3 * KT + 1], F32)
        nc.sync.dma_start(ball[:, 0:KT], b1[:])
        nc.sync.dma_start(ball[:, KT:2 * KT], b2[:])
        nc.sync.dma_start(ball[:, 2 * KT:3 * KT], b3[:])
        nc.sync.dma_start(ball[:D, 3 * KT:3 * KT + 1], b4[:])
        b1s = ball[:, 0:KT]
        b2s = ball[:, KT:2 * KT]
        b3s = ball[:, 2 * KT:3 * KT]
        b4s = ball[:D, 3 * KT:3 * KT + 1]

        # weights: DMA f32 pieces through an 8KB staging slot, cast to f32r
        w1r = wpool.tile([128, H], F32R)
        w2r = wpool.tile([128, KT, H], F32R)
        w3r = wpool.tile([128, KT, H], F32R)
        w4r = wpool.tile([128, KT, D], F32R)

        t1 = wstage.tile([128, 1, H], F32, tag="wtmp", name="wt_w1")
        nc.sync.dma_start(t1[:, 0, :], w1[:])
        nc.vector.tensor_copy(w1r[:], t1[:, 0, :])
        t4 = wstage.tile([128, KT, D], F32, tag="wtmp", name="wt_w4")
        nc.sync.dma_start(t4[:], w4[:])
        nc.vector.tensor_copy(w4r[:], t4[:])

        def load_pieces(dram3, dst, nm):
            for p in range(KT // 2):
                tmp = wstage.tile([128, 2, H], F32, tag="wtmp", name=f"wt_{nm}_{p}")
                nc.sync.dma_start(tmp[:], dram3[:, 2 * p:2 * p + 2, :])
                nc.vector.tensor_copy(dst[:, 2 * p:2 * p + 2, :], tmp[:])

        load_pieces(w2, w2r, "w2")
        load_pieces(w3, w3r, "w3")

        # persistent activation buffers (fixed roles)
        hA = wpool.tile([128, KT, NB], F32R)   # L1 out
        hB = wpool.tile([128, KT, NB], F32R)   # L2 out
        hC = wpool.tile([128, KT, NB], F32R)   # L3 out
        oT = wpool.tile([D, NB], F32)          # L4 out
        xball = wpool.tile([128, 2, KT, D], F32)
        oball = wpool.tile([128, 2, KT, D], F32)

        def dma_in(c, it=0):
            nc.sync.dma_start(xball[:, c % 2], xv[c])

        def transposes_in(c, it=0):
            xT = (xT0, xT1)[c % 2]
            for j in range(KT):
                tp = tpp.tile([D, 128], F32, tag="tp", name=f"tp_{it}_{c}_{j}")
                nc.tensor.transpose(tp[:], xball[:, c % 2, j, :], ident[:])
                nc.vector.tensor_copy(xT[:D, j * 128:(j + 1) * 128], tp[:])

        def l1_group(c, m, it=0):
            xT = (xT0, xT1)[c % 2]
            for n in range(NT):
                pm = mps.tile([128, 512], F32, tag=f"mm{n}", name=f"p1_{it}_{c}_{m}_{n}")
                nc.tensor.matmul(pm[:], w1r[:, m * 128:(m + 1) * 128],
                                 xT[:, n * 512:(n + 1) * 512], start=True, stop=True)
                nc.scalar.activation(hA[:, m, n * 512:(n + 1) * 512], pm[:],
                                     SILU, bias=b1s[:, m:m + 1])

        def mid_group(li, wr, bs, hs, hd, c, m, it=0):
            pms = [mps.tile([128, 512], F32, tag=f"mm{n}",
                            name=f"p{li}_{it}_{c}_{m}_{n}") for n in range(NT)]
            for k in range(KT):
                for n in range(NT):
                    nc.tensor.matmul(pms[n][:], wr[:, k, m * 128:(m + 1) * 128],
                                     hs[:, k, n * 512:(n + 1) * 512],
                                     start=(k == 0), stop=(k == KT - 1))
            for n in range(NT):
                nc.scalar.activation(hd[:, m, n * 512:(n + 1) * 512], pms[n][:],
                                     SILU, bias=bs[:, m:m + 1])

        def tail(c, it=0):
            # L4: hC -> oT, then transpose to batch-major and store
            for n in range(NT):
                p4 = mps.tile([D, 512], F32, tag=f"mm{n}", name=f"p4_{it}_{c}_{n}")
                for k in range(KT):
                    nc.tensor.matmul(p4[:], w4r[:, k, :],
                                     hC[:, k, n * 512:(n + 1) * 512],
                                     start=(k == 0), stop=(k == KT - 1))
                nc.vector.tensor_scalar_add(oT[:, n * 512:(n + 1) * 512], p4[:], b4s)
            for j in range(KT):
                tq = tpp.tile([128, D], F32, tag="tp", name=f"tq_{it}_{c}_{j}")
                nc.tensor.transpose(tq[:], oT[:, j * 128:(j + 1) * 128], ident[:D, :D])
                nc.vector.tensor_copy(oball[:, c % 2, j, :], tq[:])
            nc.sync.dma_start(yv[c], oball[:, c % 2])

        def full_pass(it=0):
            dma_in(0, it)
            transposes_in(0, it)
            for m in range(KT):
                l1_group(0, m, it)
            for c in range(NCH):
                if c + 1 < NCH:
                    dma_in(c + 1, it)
                for m in range(KT):
                    mid_group(2, w2r, b2s, hA, hB, c, m, it)
                for m in range(KT):
                    mid_group(3, w3r, b3s, hB, hC, c, m, it)
                    if c + 1 < NCH:
                        if m == 0:
                            transposes_in(c + 1, it)
                        l1_group(c + 1, m, it)
                tail(c, it)

        if iters == 1:
            full_pass()
        else:
            with tc.For_i(0, iters, 1):
                full_pass()

    nc.compile()
    return nc


def _prep_in_maps(t, u, W1, b1, W2, b2, W3, b3, W4, b4):
    in_maps = []
    for e in range(N_ENV):
        w1p = np.zeros((128, H), np.float32)
        w1p[:D] = W1[e]
        in_maps.append({
            "x": np.ascontiguousarray(u[e::N_ENV]),
            "w1": w1p,
            "w2": np.ascontiguousarray(W2[e].reshape(KT, 128, H).transpose(1, 0, 2)),
            "w3": np.ascontiguousarray(W3[e].reshape(KT, 128, H).transpose(1, 0, 2)),
            "w4": np.ascontiguousarray(W4[e].reshape(KT, 128, D).transpose(1, 0, 2)),
            "b1": np.ascontiguousarray(b1[e].reshape(KT, 128).T),
            "b2": np.ascontiguousarray(b2[e].reshape(KT, 128).T),
            "b3": np.ascontiguousarray(b3[e].reshape(KT, 128).T),
            "b4": np.ascontiguousarray(b4[e].reshape(D, 1)),
        })
    return in_maps


_CACHED_NC = None


def kernel(t, u, W1, b1, W2, b2, W3, b3, W4, b4):
    global _CACHED_NC
    u = np.asarray(u, np.float32)
    args = [np.asarray(a, np.float32) for a in (W1, b1, W2, b2, W3, b3, W4, b4)]
    if _CACHED_NC is None:
        _CACHED_NC = build_module()
    in_maps = _prep_in_maps(None, u, *args)
    res = run_bass_kernel_spmd(_CACHED_NC, in_maps, core_ids=list(range(N_ENV)))
    out = np.empty((B * N_ENV, D), np.float32)
    for e in range(N_ENV):
        out[e::N_ENV] = res.results[e]["y"]
    return out
